# revision 18
# baseline (speedup 1.0000x reference)
"""Bahdanau attention Trainium2 kernel.

reference math (per batch b):
    z[t, u]  = sum_d feat[t, d] * w1[u, d] + w1_b[u] + (hidden @ w2.T)[u] + w2_b[u]
    score[t] = sum_u v[u] * tanh(z[t, u]) + v_b
    attn     = softmax_t(score)
    ctx[d]   = sum_t attn[t] * feat[t, d]

Sharding: data-parallel over batch, 8 batches per core, params replicated.
Features are uploaded per-core transposed to [NB, 2, 128, T] (d on SBUF
partitions) so the w1 contraction (over d) needs no on-device transpose,
and cast to bf16 (fp32 matmuls run at half PE rate via LOW_HIGH two-pass
mode and fp32 LDWEIGHTS can't use fast-weight-load; bf16 also halves HBM
traffic). PSUM accumulation stays fp32, and the softmax chain
(exp/sum/reciprocal/normalize) runs in fp32.

Per core the pipeline is:
  - tiny fp32 matmul: cT[u, b] = w2.T-aug @ hidden-aug (biases folded in)
  - per (batch, 1024-wide t-tile):
      PE:  z_psum[128u, 1024t] = w1T.T @ featT        (bf16, 2 u x 2 d)
      ACT: tanh_sb(bf16) = tanh(z_psum + cT[:, b])    (per-partition bias)
      PE:  s_psum[1, 1024t] += vT.T @ tanh_sb          (reduce over u)
      ACT: ex(f32) = exp(s_psum + v_b), accum_out -> running sum over t
  - DVE: rec = 1/sum; attn = ex * rec (f32, the attn output) + bf16 copy
  - PE:  ab_psum[128, 1024] = ones.T @ attn_bf16       (rank-1 broadcast)
  - DVE: scalar_tensor_tensor(ft * ab) accum -> ctx[128d, 1] per d-chunk
"""

import numpy as np

B, T, D, U = 64, 2048, 256, 256
NCORES = 8
NB = B // NCORES  # batches per core
P = 128

_BUILD_CACHE = {}


def build_nc(nb=NB, t=T, v_b=0.0):
    """Build the Bass program (same program for all cores)."""
    from contextlib import ExitStack

    import concourse.bass as bass
    import concourse.tile as tile
    from concourse import bacc, mybir

    f32 = mybir.dt.float32
    bf16 = mybir.dt.bfloat16
    AF = mybir.ActivationFunctionType
    ALU = mybir.AluOpType

    ST = min(1024, t)    # t super-tile (z/tanh/score/exp granularity)
    nst = t // ST
    H = t // 2           # half-T for the attn-bcast/ctx stage
    MF = 512             # max moving free dim per fp32-psum-bank matmul

    nc = bacc.Bacc("TRN2", target_bir_lowering=False, debug=False)

    featT_d = nc.dram_tensor("featT", [nb, 2, P, t], bf16, kind="ExternalInput")
    w1T_d = nc.dram_tensor("w1T", [2, P, U], bf16, kind="ExternalInput")
    w2T_d = nc.dram_tensor("w2T", [2, P, U], f32, kind="ExternalInput")
    bsum_d = nc.dram_tensor("bsum", [1, U], f32, kind="ExternalInput")
    hT_d = nc.dram_tensor("hT", [2, P, nb], f32, kind="ExternalInput")
    v96_d = nc.dram_tensor("v96", [P, 576], bf16, kind="ExternalInput")
    ctx_d = nc.dram_tensor("ctx", [2, P, nb], f32, kind="ExternalOutput")
    abf_d = nc.dram_tensor("abf_scratch", [nb, t], bf16)
    attn_d = nc.dram_tensor("attn", [nb, t], f32, kind="ExternalOutput")

    with tile.TileContext(nc) as tc, ExitStack() as es:
        const = es.enter_context(tc.tile_pool(name="const", bufs=1))
        featp = es.enter_context(tc.tile_pool(name="feat", bufs=2 * nb))
        thp = es.enter_context(tc.tile_pool(name="th", bufs=8))
        exp_p = es.enter_context(tc.tile_pool(name="exp", bufs=3))
        smlp = es.enter_context(tc.tile_pool(name="sml", bufs=4))
        rrp = es.enter_context(tc.tile_pool(name="rr", bufs=2))
        scrp = es.enter_context(tc.tile_pool(name="scr", bufs=2))
        absp = es.enter_context(tc.tile_pool(name="absb", bufs=4))
        zps = es.enter_context(
            tc.tile_pool(name="zps", bufs=3, space=bass.MemorySpace.PSUM))
        sps = es.enter_context(
            tc.tile_pool(name="sps", bufs=2, space=bass.MemorySpace.PSUM))

        # ---- first batch's features first (head-latency), then params -----
        ft = [[None, None] for _ in range(nb)]
        for dc in range(2):
            tl = featp.tile([P, t], bf16, tag="ft", name=f"ft_0_{dc}")
            nc.sync.dma_start(tl[:], featT_d[0, dc])
            ft[0][dc] = tl

        # ---- params -> SBUF ------------------------------------------------
        w1sb, w2sb, hsb = [], [], []
        for dc in range(2):
            w1sb.append(const.tile([P, U], bf16, tag=f"w1_{dc}",
                                   name=f"w1_{dc}"))
            nc.sync.dma_start(w1sb[dc][:], w1T_d[dc])
            w2sb.append(const.tile([P, U], f32, tag=f"w2_{dc}",
                                   name=f"w2_{dc}"))
            nc.sync.dma_start(w2sb[dc][:], w2T_d[dc])
            hsb.append(const.tile([P, nb], f32, tag=f"h_{dc}", name=f"h_{dc}"))
            nc.sync.dma_start(hsb[dc][:], hT_d[dc])
        bsumsb = const.tile([1, U], f32, tag="bsum")
        nc.sync.dma_start(bsumsb[:], bsum_d[:])
        v96sb = const.tile([P, 576], bf16, tag="v96")
        nc.sync.dma_start(v96sb[:], v96_d[:])

        ones_nb = const.tile([1, nb], f32, tag="ones_nb")
        nc.vector.memset(ones_nb[:], 1.0)
        vbias = const.tile([96, 1], f32, tag="vbias")
        nc.vector.memset(vbias[:], float(v_b))

        # ---- features -> SBUF (resident) ----------------------------------
        for b in range(1, nb):
            for dc in range(2):
                tl = featp.tile([P, t], bf16, tag="ft", name=f"ft_{b}_{dc}")
                nc.sync.dma_start(tl[:], featT_d[b, dc])
                ft[b][dc] = tl

        # ---- cT[u, b] = w2T-aug @ hT-aug (fp32, exact) --------------------
        ctsb = const.tile([P, 2 * nb], f32, tag="ct")
        for uc in range(2):
            cps = zps.tile([P, nb], f32, tag="z", name=f"cps_{uc}")
            nc.tensor.matmul(cps[:], w2sb[0][:, uc * P:(uc + 1) * P], hsb[0][:],
                             start=True, stop=False)
            nc.tensor.matmul(cps[:], w2sb[1][:, uc * P:(uc + 1) * P], hsb[1][:],
                             start=False, stop=False)
            nc.tensor.matmul(cps[:], bsumsb[0:1, uc * P:(uc + 1) * P],
                             ones_nb[:], start=False, stop=True)
            nc.vector.tensor_copy(ctsb[:, uc * nb:(uc + 1) * nb], cps[:])

        ctxsb = const.tile([P, 2 * nb], f32, tag="ctx")

        # PE warmup: ~5us of throwaway matmuls on already-landed params so the
        # HAM clock-gate reaches K=8/8 (2.4 GHz) before the real z matmuls;
        # runs while the feature DMAs stream in, so it costs no wall time.
        wps = zps.tile([P, U], f32, tag="z", name="warm_ps")
        for w in range(20):
            nc.tensor.matmul(wps[:], w1sb[w % 2][:, 0:P], w1sb[(w + 1) % 2][:],
                             start=True, stop=True)

        # ---- main loop over groups of <=3 batches -------------------------
        # Score matmuls (M=1) can only target PSUM base partitions 0/32/64,
        # so up to 3 batches share one [96, 512] score tile; exp / sum /
        # normalize / bf16-cast then run once per group instead of per batch
        # (single-partition ACT/DVE ops are free-size-bound, so grouping
        # divides their cost by the group size).
        sizes = [1, 2, 3, 2] if nb == 8 else None
        if sizes is None:
            groups = [list(range(g, min(g + 3, nb))) for g in range(0, nb, 3)]
        else:
            groups, at = [], 0
            for sz in sizes:
                groups.append(list(range(at, at + sz)))
                at += sz

        def phase_a(gi, bs, c_queue):
            ex = exp_p.tile([96, t], f32, tag="ex", name=f"ex_{gi}")
            sa = smlp.tile([96, 2 * nst], f32, tag="sa", name=f"sa_{gi}")
            th = {}
            for st in range(nst):
                s0 = st * ST
                for i, b in enumerate(bs):
                    for uc in range(2):
                        zt = zps.tile([P, ST], f32, tag="z",
                                      name=f"z_{b}_{st}_{uc}")
                        for dc in range(2):
                            for q0 in range(0, ST, MF):
                                nc.tensor.matmul(
                                    zt[:, q0:q0 + MF],
                                    w1sb[dc][:, uc * P:(uc + 1) * P],
                                    ft[b][dc][:, s0 + q0:s0 + q0 + MF],
                                    start=(dc == 0), stop=(dc == 1))
                        tht = thp.tile([P, ST], bf16, tag="th",
                                       name=f"th_{b}_{st}_{uc}")
                        nc.scalar.activation(
                            tht[:], zt[:], AF.Tanh,
                            bias=ctsb[:, uc * nb + b:uc * nb + b + 1])
                        th[(b, uc)] = tht
                for q0 in range(0, ST, MF):
                    stile = sps.tile([96, MF], f32, tag="s",
                                     name=f"s_{gi}_{st}_{q0}")
                    n_mm = 2 * len(bs)
                    k = 0
                    for i, b in enumerate(bs):
                        for uc in range(2):
                            blk = (uc * 3 + i) * 96
                            nc.tensor.matmul(
                                stile[:, :], v96sb[:, blk:blk + 96],
                                th[(b, uc)][:, q0:q0 + MF],
                                start=(k == 0), stop=(k == n_mm - 1))
                            k += 1
                    nc.scalar.activation(
                        ex[:, s0 + q0:s0 + q0 + MF], stile[:], AF.Exp,
                        bias=vbias[:],
                        accum_out=sa[:, 2 * st + q0 // MF:2 * st + q0 // MF + 1])
                if c_queue:
                    c_queue.pop(0)()
            se = smlp.tile([96, 1], f32, tag="se", name=f"se_{gi}")
            nc.vector.reduce_sum(se[:], sa[:], axis=mybir.AxisListType.X)
            rec = smlp.tile([96, 1], f32, tag="rec", name=f"rec_{gi}")
            nc.vector.reciprocal(rec[:], se[:])
            return ex, rec

        def phase_b(gi, bs, ex, rec):
            asb = rrp.tile([96, t], f32, tag="asb", name=f"asb_{gi}")
            nc.vector.tensor_scalar_mul(asb[:], ex[:], rec[:, 0:1])
            abf = rrp.tile([96, t], bf16, tag="abf", name=f"abf_{gi}")
            nc.vector.tensor_copy(abf[:], asb[:])
            abt = {}
            for i, b in enumerate(bs):
                r = 32 * i
                nc.sync.dma_start(attn_d[b:b + 1, :], asb[r:r + 1, :])
                # broadcast the attn row across all 128 partitions (bf16) via
                # a DRAM bounce (DMA partition-broadcast needs a DRAM source)
                nc.sync.dma_start(abf_d[b:b + 1, :], abf[r:r + 1, :])
                ab = absp.tile([P, t], bf16, tag="ab", name=f"ab_{b}")
                nc.sync.dma_start(ab[:], abf_d[b:b + 1, :].to_broadcast((P, t)))
                abt[b] = ab
            return abt

        def phase_c_batch(b, ab):
            for dc in range(2):
                scr = scrp.tile([P, t], bf16, tag="scr", name=f"scr_{b}_{dc}")
                nc.vector.scalar_tensor_tensor(
                    out=scr[:], in0=ft[b][dc][:], scalar=1.0, in1=ab[:],
                    op0=ALU.mult, op1=ALU.mult,
                    accum_out=ctxsb[:, dc * nb + b:dc * nb + b + 1])

        # A(g) emits its t-super-tiles with one pending C batch (from the
        # previous group) interleaved after each, so the context DVE work
        # overlaps the next group's matmuls instead of serializing at the end.
        c_queue = []
        for gi, bs in enumerate(groups):
            ex, rec = phase_a(gi, bs, c_queue)
            while c_queue:
                c_queue.pop(0)()
            abt = phase_b(gi, bs, ex, rec)
            for b in bs:
                c_queue.append(lambda b=b, a=abt[b]: phase_c_batch(b, a))
        while c_queue:
            c_queue.pop(0)()

        for dc in range(2):
            nc.sync.dma_start(ctx_d[dc], ctxsb[:, dc * nb:(dc + 1) * nb])

    nc.compile()
    return nc


def prep_core_inputs(features_c, hidden_c, w1_w, w1_b, w2_w, w2_b, v_w):
    """Host-side layout prep for one core's shard (layout/dtype transforms)."""
    import ml_dtypes

    bf16 = ml_dtypes.bfloat16
    nb = features_c.shape[0]
    featT = np.ascontiguousarray(features_c.transpose(0, 2, 1)).reshape(
        nb, 2, P, -1)
    w1T = np.ascontiguousarray(w1_w.T).reshape(2, P, U)
    w2T = np.ascontiguousarray(w2_w.T).reshape(2, P, U)
    bsum = (w1_b + w2_b).reshape(1, U).astype(np.float32)
    hT = np.ascontiguousarray(hidden_c.T).reshape(2, P, nb)
    v96 = np.zeros((P, 2, 3, 96), dtype=np.float32)
    vr = v_w.reshape(2, P)
    for uc in range(2):
        for i in range(3):
            v96[:, uc, i, 32 * i] = vr[uc]
    v96 = v96.reshape(P, 576)
    return {
        "featT": featT.astype(bf16),
        "w1T": w1T.astype(bf16),
        "w2T": w2T.astype(np.float32),
        "bsum": bsum,
        "hT": hT.astype(np.float32),
        "v96": v96.astype(bf16),
    }


def kernel(features, hidden, w1_w, w1_b, w2_w, w2_b, v_w, v_b, _trace=False):
    from concourse.bass_utils import run_bass_kernel_spmd

    features = np.asarray(features, dtype=np.float32)
    hidden = np.asarray(hidden, dtype=np.float32)
    w1_w = np.asarray(w1_w, dtype=np.float32)
    w1_b = np.asarray(w1_b, dtype=np.float32)
    w2_w = np.asarray(w2_w, dtype=np.float32)
    w2_b = np.asarray(w2_b, dtype=np.float32)
    v_w = np.asarray(v_w, dtype=np.float32)
    vb = float(np.asarray(v_b).reshape(-1)[0])

    key = ("full", NB, T, vb)
    if key not in _BUILD_CACHE:
        _BUILD_CACHE[key] = build_nc(NB, T, vb)
    nc = _BUILD_CACHE[key]

    in_maps = []
    for c in range(NCORES):
        sl = slice(c * NB, (c + 1) * NB)
        in_maps.append(prep_core_inputs(
            features[sl], hidden[sl], w1_w, w1_b, w2_w, w2_b, v_w))

    res = run_bass_kernel_spmd(nc, in_maps, list(range(NCORES)), trace=_trace)

    context = np.empty((B, D), dtype=np.float32)
    attn = np.empty((B, T, 1), dtype=np.float32)
    for c in range(NCORES):
        r = res.results[c]
        # ctx [2, 128, nb] -> [nb, 256]
        context[c * NB:(c + 1) * NB] = (
            r["ctx"].transpose(2, 0, 1).reshape(NB, D))
        attn[c * NB:(c + 1) * NB] = r["attn"][..., None]
    kernel._last_exec_ns = res.exec_time_ns
    kernel._last_results = res
    return context, attn


# revision 19
# speedup vs baseline: 1.0212x; 1.0212x over previous
"""Bahdanau attention Trainium2 kernel.

reference math (per batch b):
    z[t, u]  = sum_d feat[t, d] * w1[u, d] + w1_b[u] + (hidden @ w2.T)[u] + w2_b[u]
    score[t] = sum_u v[u] * tanh(z[t, u]) + v_b
    attn     = softmax_t(score)
    ctx[d]   = sum_t attn[t] * feat[t, d]

Sharding: data-parallel over batch, 8 batches per core, params replicated.
Features are uploaded per-core transposed to [NB, 2, 128, T] (d on SBUF
partitions) so the w1 contraction (over d) needs no on-device transpose,
and cast to bf16 (fp32 matmuls run at half PE rate via LOW_HIGH two-pass
mode and fp32 LDWEIGHTS can't use fast-weight-load; bf16 also halves HBM
traffic). PSUM accumulation stays fp32, and the softmax chain
(exp/sum/reciprocal/normalize) runs in fp32.

Per core the pipeline is:
  - tiny fp32 matmul: cT[u, b] = w2.T-aug @ hidden-aug (biases folded in)
  - per (batch, 1024-wide t-tile):
      PE:  z_psum[128u, 1024t] = w1T.T @ featT        (bf16, 2 u x 2 d)
      ACT: tanh_sb(bf16) = tanh(z_psum + cT[:, b])    (per-partition bias)
      PE:  s_psum[1, 1024t] += vT.T @ tanh_sb          (reduce over u)
      ACT: ex(f32) = exp(s_psum + v_b), accum_out -> running sum over t
  - DVE: rec = 1/sum; attn = ex * rec (f32, the attn output) + bf16 copy
  - PE:  ab_psum[128, 1024] = ones.T @ attn_bf16       (rank-1 broadcast)
  - DVE: scalar_tensor_tensor(ft * ab) accum -> ctx[128d, 1] per d-chunk
"""

import numpy as np

B, T, D, U = 64, 2048, 256, 256
NCORES = 8
NB = B // NCORES  # batches per core
P = 128

_BUILD_CACHE = {}


def build_nc(nb=NB, t=T, v_b=0.0):
    """Build the Bass program (same program for all cores)."""
    from contextlib import ExitStack

    import concourse.bass as bass
    import concourse.tile as tile
    from concourse import bacc, mybir

    f32 = mybir.dt.float32
    bf16 = mybir.dt.bfloat16
    AF = mybir.ActivationFunctionType
    ALU = mybir.AluOpType

    ST = min(1024, t)    # t super-tile (z/tanh/score/exp granularity)
    nst = t // ST
    H = t // 2           # half-T for the attn-bcast/ctx stage
    MF = 512             # max moving free dim per fp32-psum-bank matmul

    nc = bacc.Bacc("TRN2", target_bir_lowering=False, debug=False)

    featT_d = nc.dram_tensor("featT", [nb, 2, P, t], bf16, kind="ExternalInput")
    CB = 2 * U + 576                 # bf16 blob: w1T (2 chunks) | v96
    CF = 2 * U + 2 * nb + U          # f32 blob: w2T | hT | bsum(row 0)
    pbf_d = nc.dram_tensor("pbf", [P, CB], bf16, kind="ExternalInput")
    pf32_d = nc.dram_tensor("pf32", [P, CF], f32, kind="ExternalInput")
    ctx_d = nc.dram_tensor("ctx", [2, P, nb], f32, kind="ExternalOutput")
    abf_d = nc.dram_tensor("abf_scratch", [nb, t], bf16)
    attn_d = nc.dram_tensor("attn", [nb, t], f32, kind="ExternalOutput")

    with tile.TileContext(nc) as tc, ExitStack() as es:
        const = es.enter_context(tc.tile_pool(name="const", bufs=1))
        featp = es.enter_context(tc.tile_pool(name="feat", bufs=nb))
        thp = es.enter_context(tc.tile_pool(name="th", bufs=8))
        exp_p = es.enter_context(tc.tile_pool(name="exp", bufs=3))
        smlp = es.enter_context(tc.tile_pool(name="sml", bufs=4))
        rrp = es.enter_context(tc.tile_pool(name="rr", bufs=2))
        scrp = es.enter_context(tc.tile_pool(name="scr", bufs=2))
        absp = es.enter_context(tc.tile_pool(name="absb", bufs=4))
        zps = es.enter_context(
            tc.tile_pool(name="zps", bufs=3, space=bass.MemorySpace.PSUM))
        sps = es.enter_context(
            tc.tile_pool(name="sps", bufs=2, space=bass.MemorySpace.PSUM))

        # ---- params -> SBUF (two packed blobs, issued before features) ----
        pbf = const.tile([P, CB], bf16, tag="pbf")
        nc.sync.dma_start(pbf[:], pbf_d[:])
        pf32 = const.tile([P, CF], f32, tag="pf32")
        nc.sync.dma_start(pf32[:], pf32_d[:])
        w1sb = [pbf[:, dc * U:(dc + 1) * U] for dc in range(2)]
        v96sb = pbf[:, 2 * U:2 * U + 576]
        w2sb = [pf32[:, dc * U:(dc + 1) * U] for dc in range(2)]
        hsb = [pf32[:, 2 * U + dc * nb:2 * U + (dc + 1) * nb] for dc in range(2)]
        bsumsb = pf32[0:1, 2 * U + 2 * nb:2 * U + 2 * nb + U]

        ones_nb = const.tile([1, nb], f32, tag="ones_nb")
        nc.vector.memset(ones_nb[:], 1.0)
        vbias = const.tile([96, 1], f32, tag="vbias")
        nc.vector.memset(vbias[:], float(v_b))

        # ---- features -> SBUF (resident; one merged DMA per batch) --------
        ft = [None] * nb
        for b in range(nb):
            tl = featp.tile([P, 2, t], bf16, tag="ft", name=f"ft_{b}")
            nc.sync.dma_start(tl[:], featT_d[b].rearrange("dc p t -> p dc t"))
            ft[b] = [tl[:, 0, :], tl[:, 1, :]]

        # ---- cT[u, b] = w2T-aug @ hT-aug (fp32, exact) --------------------
        ctsb = const.tile([P, 2 * nb], f32, tag="ct")
        for uc in range(2):
            cps = zps.tile([P, nb], f32, tag="z", name=f"cps_{uc}")
            nc.tensor.matmul(cps[:], w2sb[0][:, uc * P:(uc + 1) * P], hsb[0],
                             start=True, stop=False)
            nc.tensor.matmul(cps[:], w2sb[1][:, uc * P:(uc + 1) * P], hsb[1],
                             start=False, stop=False)
            nc.tensor.matmul(cps[:], bsumsb[:, uc * P:(uc + 1) * P],
                             ones_nb[:], start=False, stop=True)
            nc.vector.tensor_copy(ctsb[:, uc * nb:(uc + 1) * nb], cps[:])

        ctxsb = const.tile([P, 2 * nb], f32, tag="ctx")

        # PE warmup: ~5us of throwaway matmuls on already-landed params so the
        # HAM clock-gate reaches K=8/8 (2.4 GHz) before the real z matmuls;
        # runs while the feature DMAs stream in, so it costs no wall time.
        wps = zps.tile([P, U], f32, tag="z", name="warm_ps")
        for w in range(16):
            nc.tensor.matmul(wps[:], w1sb[w % 2][:, 0:P], w1sb[(w + 1) % 2],
                             start=True, stop=True)

        # ---- main loop over groups of <=3 batches -------------------------
        # Score matmuls (M=1) can only target PSUM base partitions 0/32/64,
        # so up to 3 batches share one [96, 512] score tile; exp / sum /
        # normalize / bf16-cast then run once per group instead of per batch
        # (single-partition ACT/DVE ops are free-size-bound, so grouping
        # divides their cost by the group size).
        sizes = [1, 2, 3, 2] if nb == 8 else None
        if sizes is None:
            groups = [list(range(g, min(g + 3, nb))) for g in range(0, nb, 3)]
        else:
            groups, at = [], 0
            for sz in sizes:
                groups.append(list(range(at, at + sz)))
                at += sz

        def phase_a(gi, bs, c_queue):
            ex = exp_p.tile([96, t], f32, tag="ex", name=f"ex_{gi}")
            sa = smlp.tile([96, 2 * nst], f32, tag="sa", name=f"sa_{gi}")
            th = {}
            for st in range(nst):
                s0 = st * ST
                for i, b in enumerate(bs):
                    for uc in range(2):
                        zt = zps.tile([P, ST], f32, tag="z",
                                      name=f"z_{b}_{st}_{uc}")
                        for dc in range(2):
                            for q0 in range(0, ST, MF):
                                nc.tensor.matmul(
                                    zt[:, q0:q0 + MF],
                                    w1sb[dc][:, uc * P:(uc + 1) * P],
                                    ft[b][dc][:, s0 + q0:s0 + q0 + MF],
                                    start=(dc == 0), stop=(dc == 1))
                        tht = thp.tile([P, ST], bf16, tag="th",
                                       name=f"th_{b}_{st}_{uc}")
                        nc.scalar.activation(
                            tht[:], zt[:], AF.Tanh,
                            bias=ctsb[:, uc * nb + b:uc * nb + b + 1])
                        th[(b, uc)] = tht
                for q0 in range(0, ST, MF):
                    stile = sps.tile([96, MF], f32, tag="s",
                                     name=f"s_{gi}_{st}_{q0}")
                    n_mm = 2 * len(bs)
                    k = 0
                    for i, b in enumerate(bs):
                        for uc in range(2):
                            blk = (uc * 3 + i) * 96
                            nc.tensor.matmul(
                                stile[:, :], v96sb[:, blk:blk + 96],
                                th[(b, uc)][:, q0:q0 + MF],
                                start=(k == 0), stop=(k == n_mm - 1))
                            k += 1
                    nc.scalar.activation(
                        ex[:, s0 + q0:s0 + q0 + MF], stile[:], AF.Exp,
                        bias=vbias[:],
                        accum_out=sa[:, 2 * st + q0 // MF:2 * st + q0 // MF + 1])
                if c_queue:
                    c_queue.pop(0)()
            se = smlp.tile([96, 1], f32, tag="se", name=f"se_{gi}")
            nc.vector.reduce_sum(se[:], sa[:], axis=mybir.AxisListType.X)
            rec = smlp.tile([96, 1], f32, tag="rec", name=f"rec_{gi}")
            nc.vector.reciprocal(rec[:], se[:])
            return ex, rec

        def phase_b(gi, bs, ex, rec):
            asb = rrp.tile([96, t], f32, tag="asb", name=f"asb_{gi}")
            nc.vector.tensor_scalar_mul(asb[:], ex[:], rec[:, 0:1])
            abf = rrp.tile([96, t], bf16, tag="abf", name=f"abf_{gi}")
            nc.vector.tensor_copy(abf[:], asb[:])
            abt = {}
            for i, b in enumerate(bs):
                r = 32 * i
                nc.sync.dma_start(attn_d[b:b + 1, :], asb[r:r + 1, :])
                # broadcast the attn row across all 128 partitions (bf16) via
                # a DRAM bounce (DMA partition-broadcast needs a DRAM source)
                nc.sync.dma_start(abf_d[b:b + 1, :], abf[r:r + 1, :])
                ab = absp.tile([P, t], bf16, tag="ab", name=f"ab_{b}")
                nc.sync.dma_start(ab[:], abf_d[b:b + 1, :].to_broadcast((P, t)))
                abt[b] = ab
            return abt

        def phase_c_batch(b, ab):
            for dc in range(2):
                scr = scrp.tile([P, t], bf16, tag="scr", name=f"scr_{b}_{dc}")
                nc.vector.scalar_tensor_tensor(
                    out=scr[:], in0=ft[b][dc][:], scalar=1.0, in1=ab[:],
                    op0=ALU.mult, op1=ALU.mult,
                    accum_out=ctxsb[:, dc * nb + b:dc * nb + b + 1])

        # A(g) emits its t-super-tiles with one pending C batch (from the
        # previous group) interleaved after each, so the context DVE work
        # overlaps the next group's matmuls instead of serializing at the end.
        c_queue = []
        for gi, bs in enumerate(groups):
            ex, rec = phase_a(gi, bs, c_queue)
            while c_queue:
                c_queue.pop(0)()
            abt = phase_b(gi, bs, ex, rec)
            for b in bs:
                c_queue.append(lambda b=b, a=abt[b]: phase_c_batch(b, a))
        while c_queue:
            c_queue.pop(0)()

        for dc in range(2):
            nc.sync.dma_start(ctx_d[dc], ctxsb[:, dc * nb:(dc + 1) * nb])

    nc.compile()
    return nc


def prep_core_inputs(features_c, hidden_c, w1_w, w1_b, w2_w, w2_b, v_w):
    """Host-side layout prep for one core's shard (layout/dtype transforms)."""
    import ml_dtypes

    bf16 = ml_dtypes.bfloat16
    nb = features_c.shape[0]
    featT = np.ascontiguousarray(features_c.transpose(0, 2, 1)).reshape(
        nb, 2, P, -1)
    w1T = np.ascontiguousarray(w1_w.T).reshape(2, P, U)
    w2T = np.ascontiguousarray(w2_w.T).reshape(2, P, U)
    bsum = (w1_b + w2_b).reshape(1, U).astype(np.float32)
    hT = np.ascontiguousarray(hidden_c.T).reshape(2, P, nb)
    v96 = np.zeros((P, 2, 3, 96), dtype=np.float32)
    vr = v_w.reshape(2, P)
    for uc in range(2):
        for i in range(3):
            v96[:, uc, i, 32 * i] = vr[uc]
    v96 = v96.reshape(P, 576)
    pbf = np.concatenate([w1T[0], w1T[1], v96], axis=1)
    pf32 = np.zeros((P, 2 * U + 2 * nb + U), dtype=np.float32)
    pf32[:, 0:U] = w2T[0]
    pf32[:, U:2 * U] = w2T[1]
    pf32[:, 2 * U:2 * U + nb] = hT[0]
    pf32[:, 2 * U + nb:2 * U + 2 * nb] = hT[1]
    pf32[0, 2 * U + 2 * nb:] = bsum[0]
    return {
        "featT": featT.astype(bf16),
        "pbf": pbf.astype(bf16),
        "pf32": pf32,
    }


def kernel(features, hidden, w1_w, w1_b, w2_w, w2_b, v_w, v_b, _trace=False):
    from concourse.bass_utils import run_bass_kernel_spmd

    features = np.asarray(features, dtype=np.float32)
    hidden = np.asarray(hidden, dtype=np.float32)
    w1_w = np.asarray(w1_w, dtype=np.float32)
    w1_b = np.asarray(w1_b, dtype=np.float32)
    w2_w = np.asarray(w2_w, dtype=np.float32)
    w2_b = np.asarray(w2_b, dtype=np.float32)
    v_w = np.asarray(v_w, dtype=np.float32)
    vb = float(np.asarray(v_b).reshape(-1)[0])

    key = ("full", NB, T, vb)
    if key not in _BUILD_CACHE:
        _BUILD_CACHE[key] = build_nc(NB, T, vb)
    nc = _BUILD_CACHE[key]

    in_maps = []
    for c in range(NCORES):
        sl = slice(c * NB, (c + 1) * NB)
        in_maps.append(prep_core_inputs(
            features[sl], hidden[sl], w1_w, w1_b, w2_w, w2_b, v_w))

    res = run_bass_kernel_spmd(nc, in_maps, list(range(NCORES)), trace=_trace)

    context = np.empty((B, D), dtype=np.float32)
    attn = np.empty((B, T, 1), dtype=np.float32)
    for c in range(NCORES):
        r = res.results[c]
        # ctx [2, 128, nb] -> [nb, 256]
        context[c * NB:(c + 1) * NB] = (
            r["ctx"].transpose(2, 0, 1).reshape(NB, D))
        attn[c * NB:(c + 1) * NB] = r["attn"][..., None]
    kernel._last_exec_ns = res.exec_time_ns
    kernel._last_results = res
    return context, attn
